# revision 22
# baseline (speedup 1.0000x reference)
"""Trainium2 Bass kernel for ContextHyperLinearSSM.

Computes out[b,:] = x[b,:] @ (WA[context[b]] * adj_xx) + u[b,:] @ (WB[context[b]] * adj_xu)

Strategy: shard the CONTEXT axis across the 8 cores (64 contexts each).
The host groups samples by context, masks the weight banks with the
adjacency masks, and quantizes the weights to fp8-e3m4 (x2^6 scale; the
inverse 2^-6 is folded into the bf16 activations — both scalings are exact
powers of two, so the only quantization error is the e3m4 weight rounding,
measured at 1.3e-2 absmax/scale against the fp32 reference).  Each core
streams its 64 contexts' weights from HBM exactly once and runs 3
accumulating mixed-dtype matmuls (bf16 stationary x fp8 moving) per
context.  Each sample's row is computed by exactly one core, so the
host-side unshard is a pure scatter.

Contexts are globally sorted by sample count and dealt round-robin to the
cores, so every core sees the same per-group padded size G_g (required:
one SPMD program serves all cores) and the padding tracks the count
distribution.  ALL DMAs are issued up front, before any dependent work
lands on the two HWDGE rings' issuing engines (sync/scalar), so the
weight stream runs back-to-back at line rate (~420 GB/s measured).
Whole-group transfers stream fastest; only group 0 (pipeline fill) and
the last group (receipt pipelining at the tail) are split into halves.

Compute: where 2*G <= 32, a context pair (ci, CH+ci) merges into ONE
matmul per k-slot — stationary [x_ci | x_pair] (2G cols), moving
[W_ci[k] | W_pair[k]] (N=512); the two cross quadrants are garbage the
host never reads.  This halves the LDWEIGHTS/MATMUL count so even a
HAM-throttled (1.2 GHz) PE outruns the weight stream.  A short burst of
dummy matmuls on a zeroed tile pre-warms the HAM clock gate during the
DMA fill phase.  Pairs cycle the four 128x32 column tiles of the PE
array; each group accumulates in one PSUM bank.  One full-width copy per
group (vector/scalar alternating; the last group split across both)
drains PSUM into a bf16 staging tile, which is flushed by a few
consolidated multi-group DMAs (the final group as two free-dim halves,
each chained to its own drain half).
"""

import sys

sys.path.insert(0, "/opt/trn_rl_repo")

import ml_dtypes
import numpy as np

import concourse.bass as bass
import concourse.mybir as mybir
import concourse.tile as tile
from concourse import bacc
from concourse.bass_utils import run_bass_kernel_spmd

N_CORES = 8
CT = 8  # contexts per PSUM group
WSCALE = 64.0  # 2^6: weights *= WSCALE (into e3m4 range), acts /= WSCALE

BF16 = ml_dtypes.bfloat16
FP8 = ml_dtypes.float8_e3m4


def _install_profile_shim():
    """Register the NTFF profile hook that trn_boot skips when
    antenv.axon_hooks is missing from the image (profiling only)."""
    import types
    if "antenv.axon_hooks" in sys.modules:
        return
    try:
        from trn_agent_boot.trn_boot import _ntff_profile_via_ctypes
        hook = _ntff_profile_via_ctypes("/opt/axon/libaxon_pjrt.so")
    except Exception:
        hook = None
    mod = types.ModuleType("antenv.axon_hooks")
    mod.get_axon_ntff_profile_hook = lambda: hook
    mod.set_axon_ntff_profile_hook = lambda h: None
    sys.modules["antenv.axon_hooks"] = mod


def _geometry(S, A, Gs):
    """Static geometry shared by host and device.

    PSUM packing: context c of a group -> bank t=c//CPT, partition slot
    sl=(c%CPT)%NSL (SLP-aligned), free half cf=(c%CPT)//NSL.
    """
    HS = S // 128
    K = HS + 1
    CH = CT // 2
    WF = CH * K * S
    FF = max(1, min(CT, 512 // S))
    SLP = 32 if max(Gs) <= 32 else 64
    NSL = 128 // SLP
    CPT = min(CT, NSL * FF)
    T = -(-CT // CPT)
    assert T * CPT == CT
    NG = len(Gs)
    # out-staging splits: [0,NG/2), [NG/2,NG-1), [NG-1,NG) — early flushes
    # plus a tiny final one (groups are sorted largest-first)
    splits = [(0, NG // 2), (NG // 2, NG - 1), (NG - 1, NG)]
    splits = [(a, b) for a, b in splits if b > a]
    off_a = 0
    offs_a = []
    for G in Gs:
        offs_a.append(off_a)
        off_a += 2 * CH * K * G
    OW = T * FF * S  # staging elems per partition line per group
    # merged pair-matmul path: context pair (ci, CH+ci) stacks into one
    # 32-partition strip (2G rows) with an N=2S moving operand; needs the
    # 4 strips (NSL==CH) and both pair members inside one strip
    mrg = [T == 1 and NSL == CH and FF == 2 and 2 * G <= SLP for G in Gs]
    # compact per-group flush via strided-partition DMA: measured broken
    # on HW (wrong data + slower descriptors) — keep the padded
    # consolidated-split flushes instead
    cpk = False
    rows_o = [(2 * G if mrg[g] else G) if cpk else 128
              for g, G in enumerate(Gs)]
    offs_o = []
    off_o = 0
    for g in range(NG):
        offs_o.append(off_o)
        off_o += (CH * rows_o[g] if cpk else 128) * OW
    return dict(HS=HS, K=K, CH=CH, WF=WF, FF=FF, SLP=SLP, NSL=NSL,
                CPT=CPT, T=T, NG=NG, splits=splits, offs_a=offs_a,
                AL=off_a, OW=OW, OL=off_o, mrg=mrg, cpk=cpk,
                rows_o=rows_o, offs_o=offs_o)


def _build_program(S, A, Gs):
    """Build the per-core Bass program for per-group sizes Gs."""
    f32 = mybir.dt.float32
    bf16 = mybir.dt.bfloat16
    fp8 = mybir.dt.float8e3
    nc = bacc.Bacc("TRN2", target_bir_lowering=False)

    geo = _geometry(S, A, Gs)
    K, CH, WF, FF = geo["K"], geo["CH"], geo["WF"], geo["FF"]
    SLP, NSL, CPT, T = geo["SLP"], geo["NSL"], geo["CPT"], geo["T"]
    NG, OW = geo["NG"], geo["OW"]
    assert S % 128 == 0 and A == 128

    wts = nc.dram_tensor("wts", [NG, 128, 2 * WF], fp8,
                         kind="ExternalInput").ap()
    acts = nc.dram_tensor("acts", [128, geo["AL"]], bf16,
                          kind="ExternalInput").ap()
    out = nc.dram_tensor("out", [geo["OL"]], bf16,
                         kind="ExternalOutput").ap()

    with tile.TileContext(nc) as tc:
        with (
            tc.tile_pool(name="a", bufs=1) as apool,
            tc.tile_pool(name="w", bufs=NG) as wpool,
            tc.tile_pool(name="o", bufs=1) as opool,
            tc.tile_pool(name="d", bufs=1) as dpool,
            tc.tile_pool(name="psum", bufs=8, space="PSUM") as psum,
        ):
            # ---- ALL DMAs issued up front, before any dependent work
            # lands on the issuing engines: the two HWDGE rings then
            # stream back-to-back at line rate with no issue stalls.
            # Every transfer is split into free-dim halves, one half per
            # ring, in group-consumption order: the rings stay byte-
            # balanced, each group's halves land as a pair right before
            # the PE needs them, and each half's completion receipt
            # (~1.5us) hides behind the next half's transfer.
            at = apool.tile([128, geo["AL"]], bf16)
            nc.scalar.dma_start(at[:], acts[:])
            # shared output staging tile, flushed by consolidated DMAs
            os_t = opool.tile([128, NG * OW], bf16)

            # whole-group transfers stream fastest (~424 GB/s vs ~395 for
            # half-splits); halves only where latency matters — group 0
            # (pipeline fill) and the last group (its second half's
            # completion receipt hides behind the first's matmuls)
            wts_t = [wpool.tile([128, 2 * WF], fp8, tag="wt", name=f"wt_{g}")
                     for g in range(NG)]
            nc.sync.dma_start(wts_t[0][:, :WF], wts[0, :, :WF])
            nc.scalar.dma_start(wts_t[0][:, WF:], wts[0, :, WF:])
            for g in range(1, NG - 1):
                ring = nc.sync if g % 2 == 1 else nc.scalar
                ring.dma_start(wts_t[g][:], wts[g])
            if geo["mrg"][NG - 1]:
                # last group as four pair-DMAs: each pair's matmuls are
                # gated by their own completion receipt, so only the
                # final pair's 3 matmuls wait for the last receipt
                PW = K * 2 * S
                for ci in range(CH):
                    r = nc.sync if ci % 2 == 0 else nc.scalar
                    r.dma_start(wts_t[NG - 1][:, ci * PW:(ci + 1) * PW],
                                wts[NG - 1, :, ci * PW:(ci + 1) * PW])
            else:
                nc.sync.dma_start(wts_t[NG - 1][:, :WF], wts[NG - 1, :, :WF])
                nc.scalar.dma_start(wts_t[NG - 1][:, WF:],
                                    wts[NG - 1, :, WF:])

            # ---- PE warm-up: the HAM clock gate holds the PE at 1.2 GHz
            # until it has been busy for a full ~3.4us window.  Dummy
            # matmuls on a zeroed tile (into the PSUM bank that group
            # NG-1 will reuse much later) run during the DMA fill phase
            # so the real matmuls start at 2.4 GHz.
            dt = dpool.tile([128, 512], bf16)
            nc.vector.memset(dt[:], 0.0)
            ps_dummy = psum.tile([128, 512], f32, tag="ps", name="ps_dummy")
            for i in range(9):
                nc.tensor.matmul(ps_dummy[:, :256], lhsT=dt[:, :128],
                                 rhs=dt[:, :256], start=True, stop=True)

            for g in range(NG):
                G = Gs[g]
                wt = wts_t[g]

                def views(c):
                    hf, ci = divmod(c, CH)
                    wv = wt[:, hf * WF + ci * K * S:
                            hf * WF + (ci + 1) * K * S] \
                        .rearrange("p (k s) -> p k s", k=K)
                    a0 = geo["offs_a"][g] + (hf * CH + ci) * K * G
                    av = at[:, a0:a0 + K * G] \
                        .rearrange("p (k g) -> p k g", k=K)
                    return wv, av

                ps_tiles = [psum.tile([128, FF * S], f32, tag="ps",
                                      name=f"ps_{g}_{t}")
                            for t in range(T)]
                if geo["mrg"][g]:
                    # merged path: context pair (ci, CH+ci) shares one
                    # matmul — stationary [x_ci | x_{CH+ci}] (2G cols),
                    # moving [W_ci[k] | W_{CH+ci}[k]] (2S cols).  The two
                    # cross quadrants of each [2G, 2S] output are garbage
                    # that the host never reads; the payoff is half the
                    # LDWEIGHTS/MATMUL instructions and N=512 streams, so
                    # even a cold (HAM-throttled) PE outruns the weight
                    # stream.  Pairs cycle the four column strips.
                    for ci in range(CH):
                        wv2 = wt[:, ci * K * 2 * S:(ci + 1) * K * 2 * S] \
                            .rearrange("p (k s) -> p k s", k=K)
                        a0 = geo["offs_a"][g] + ci * K * 2 * G
                        av2 = at[:, a0:a0 + K * 2 * G] \
                            .rearrange("p (k g) -> p k g", k=K)
                        pslice = ps_tiles[0][ci * SLP:ci * SLP + 2 * G, :]
                        for k in range(K):
                            nc.tensor.matmul(
                                pslice,
                                lhsT=av2[:, k, :],
                                rhs=wv2[:, k, :],
                                start=(k == 0), stop=(k == K - 1),
                                tile_position=(0, ci * SLP))
                else:
                    # consecutive contexts cycle the NSL column tiles of
                    # the PE array so their streams execute concurrently;
                    # each context's K accumulating matmuls stay adjacent
                    for c in range(CT):
                        wv, av = views(c)
                        t, r2 = divmod(c, CPT)
                        sl, cf = r2 % NSL, r2 // NSL
                        pslice = ps_tiles[t][sl * SLP:sl * SLP + G,
                                             cf * S:cf * S + S]
                        for k in range(K):
                            nc.tensor.matmul(
                                pslice,
                                lhsT=av[:, k, :],
                                rhs=wv[:, k, :],
                                start=(k == 0), stop=(k == K - 1),
                                tile_position=(0, sl * SLP))
                o0 = g * T * FF * S
                if g == NG - 1:
                    # tail group: split the latency-critical final drain
                    # into free-dim halves on vector+scalar in parallel
                    for t in range(T):
                        half = FF * S // 2
                        nc.vector.tensor_copy(
                            os_t[:, o0 + t * FF * S:
                                 o0 + t * FF * S + half],
                            ps_tiles[t][:, :half])
                        nc.scalar.copy(
                            os_t[:, o0 + t * FF * S + half:
                                 o0 + (t + 1) * FF * S],
                            ps_tiles[t][:, half:])
                else:
                    for t in range(T):
                        # all mid drains on vector: scalar's queue stays
                        # clear for the final group's drain half, and
                        # scalar-ring flushes below can issue early
                        nc.vector.tensor_copy(
                            os_t[:, o0 + t * FF * S:
                                 o0 + (t + 1) * FF * S],
                            ps_tiles[t][:, :])
                # early flush of a completed split, emitted as soon as
                # its last drain exists.  Split 0 rides the SCALAR ring:
                # its descriptors queue behind the last weight transfer,
                # so its writes land in the otherwise-idle HBM window
                # during the tail receipts instead of stealing read
                # bandwidth from the weight stream.  Split 1 (issued at
                # the second-last group's drain) rides sync, whose
                # weight bytes are long gone by then.
                if not geo["cpk"]:
                    for si, (a, b) in enumerate(geo["splits"][:-1]):
                        if g == b - 1:
                            ring = nc.scalar if si == 0 else nc.sync
                            dst = out[128 * a * OW: 128 * b * OW]
                            ring.dma_start(
                                dst.rearrange("(p w) -> p w", p=128),
                                os_t[:, a * OW:b * OW])
                if geo["cpk"]:
                    # compact per-group flush: only the valid strip rows
                    # ship to HBM (sync ring; issues overlap the stream)
                    R = geo["rows_o"][g]
                    src = os_t[:, o0:o0 + OW] \
                        .rearrange("(a b) w -> a b w", a=CH)[:, :R, :]
                    dst = out[geo["offs_o"][g]:
                              geo["offs_o"][g] + CH * R * OW] \
                        .rearrange("(a b w) -> a b w", a=CH, b=R)
                    nc.sync.dma_start(dst, src)

            if not geo["cpk"]:
                # final-split flush: two free-half DMAs, each chained to
                # its own drain half (mid splits were flushed inside the
                # group loop above)
                a, b = geo["splits"][-1]
                if T == 1 and FF == 2:
                    half = OW // 2
                    for cf in range(2):
                        dst = out[128 * a * OW + cf * 128 * half:
                                  128 * a * OW + (cf + 1) * 128 * half]
                        ring = nc.sync if cf == 0 else nc.scalar
                        ring.dma_start(
                            dst.rearrange("(p w) -> p w", p=128),
                            os_t[:, a * OW + cf * half:
                                 a * OW + (cf + 1) * half])
                else:
                    dst = out[128 * a * OW: 128 * b * OW]
                    nc.sync.dma_start(
                        dst.rearrange("(p w) -> p w", p=128),
                        os_t[:, a * OW:b * OW])

    nc.compile()
    return nc


def kernel(x, u, WA, WB, adj_xx, adj_xu, context, _trace=False):
    B, S = x.shape
    _, A = u.shape
    C = WA.shape[0]
    assert C % N_CORES == 0
    CP = C // N_CORES
    assert CP % CT == 0
    NG = CP // CT

    # ---- host-side shard: count-sorted contexts, dealt round-robin ----
    context = np.asarray(context)
    cnt = np.bincount(context, minlength=C)
    perm = np.argsort(-cnt, kind="stable")          # contexts by count desc
    # context at global rank r -> core r%8, position r//8; group = pos//CT.
    # All cores share one program, so G_g is set by the chunk's global max
    # count = count at rank g*CT*N_CORES.
    Gs = []
    for g in range(NG):
        m = int(cnt[perm[g * CT * N_CORES]])
        Gs.append(max(2, ((m + 1) // 2) * 2))

    geo = _geometry(S, A, Gs)
    HS, K, CH, WF = geo["HS"], geo["K"], geo["CH"], geo["WF"]
    FF, SLP, NSL, CPT, T, OW = (geo["FF"], geo["SLP"], geo["NSL"],
                                geo["CPT"], geo["T"], geo["OW"])

    order = np.argsort(context, kind="stable")
    starts = np.zeros(C + 1, np.int64)
    starts[1:] = np.cumsum(cnt)

    def group_rows(ctx_ids, G):
        """gidx [len,G] sample indices (clamped) + valid mask."""
        j = np.arange(G)
        cc = cnt[ctx_ids][:, None]
        valid = j[None, :] < cc
        pos = starts[ctx_ids][:, None] + np.minimum(j[None, :],
                                                    np.maximum(cc - 1, 0))
        return order[pos], valid

    inv = np.float32(1.0 / WSCALE)
    x = np.asarray(x, np.float32) * inv
    u = np.asarray(u, np.float32) * inv

    # pre-mask the weight banks, scale into e3m4 range, quantize on host
    Am = (np.asarray(WA, np.float32) * np.float32(WSCALE)
          * np.asarray(adj_xx, np.float32)).astype(FP8)    # [C, S, S]
    Bm = (np.asarray(WB, np.float32) * np.float32(WSCALE)
          * np.asarray(adj_xu, np.float32)).astype(FP8)    # [C, A, S]

    in_maps = []
    scatter = []   # per core: list of (ctx_ids, gidx, valid) per group
    for k in range(N_CORES):
        wblob = np.empty((NG, 128, 2 * WF), FP8)
        ablob = np.zeros((128, geo["AL"]), BF16)
        sc = []
        for g in range(NG):
            G = Gs[g]
            ctx_ids = perm[(g * CT + np.arange(CT)) * N_CORES + k]
            gidx, valid = group_rows(ctx_ids, G)           # [CT, G]
            sc.append((ctx_ids, gidx, valid))
            XpT = x[gidx].transpose(0, 2, 1).astype(BF16)  # [CT, S, G]
            UpT = u[gidx].transpose(0, 2, 1).astype(BF16)  # [CT, A, G]
            # per-partition element order: legacy (hf, ci, k, s) vs
            # merged (ci, k, hf, s); both views expose [hf,ci,128,k,s]
            if geo["mrg"][g]:
                wb = wblob[g].reshape(128, CH, K, 2, S) \
                    .transpose(3, 1, 0, 2, 4)
            else:
                wb = wblob[g].reshape(128, 2, CH, K, S) \
                    .transpose(1, 2, 0, 3, 4)
            wb[..., 0, :] = Bm[ctx_ids].reshape(2, CH, 128, S)
            wb[..., 1:, :] = Am[ctx_ids].reshape(2, CH, HS, 128, S) \
                .transpose(0, 1, 3, 2, 4)
            asl = ablob[:, geo["offs_a"][g]:
                        geo["offs_a"][g] + 2 * CH * K * G]
            if geo["mrg"][g]:
                A3 = asl.reshape(128, CH, K, 2, G).transpose(3, 1, 2, 0, 4)
            else:
                A3 = asl.reshape(128, 2, CH, K, G).transpose(1, 2, 3, 0, 4)
            A3[:, :, 0] = UpT.reshape(2, CH, 128, G)
            A3[:, :, 1:] = XpT.reshape(2, CH, HS, 128, G)
        in_maps.append({"wts": wblob, "acts": ablob})
        scatter.append(sc)

    if _trace:
        _install_profile_shim()
    nc = _build_program(S, A, Gs)
    res = run_bass_kernel_spmd(nc, in_maps, core_ids=list(range(N_CORES)),
                               trace=_trace)

    out_full = np.zeros((B, S), np.float32)
    for k, r in enumerate(res.results):
        v = np.asarray(r["out"]).astype(np.float32)
        if geo["cpk"]:
            # compact per-group blocks: [CH strips, rows_o, FF*S]
            for g in range(NG):
                R = geo["rows_o"][g]
                blk = v[geo["offs_o"][g]:
                        geo["offs_o"][g] + CH * R * geo["OW"]] \
                    .reshape(CH, R, FF, S)
                ctx_ids, gidx, valid = scatter[k][g]
                for c in range(CT):
                    sl, cf = c % NSL, c // NSL
                    rb = cf * Gs[g] if geo["mrg"][g] else 0
                    rows = blk[sl, rb:rb + Gs[g], cf, :]     # [G, S]
                    m = valid[c]
                    out_full[gidx[c][m]] = rows[m]
            continue
        for si, (a, b) in enumerate(geo["splits"]):
            nsp = len(geo["splits"])
            if si == nsp - 1 and T == 1 and FF == 2:
                half = OW // 2
                h = [v[128 * a * OW + cf * 128 * half:
                       128 * a * OW + (cf + 1) * 128 * half]
                     .reshape(128, 1, T, 1, S) for cf in range(2)]
                blk = np.concatenate(h, axis=3)   # [128, 1, T, FF, S]
            else:
                blk = v[128 * a * OW: 128 * b * OW] \
                    .reshape(128, b - a, T, FF, S)
            for g in range(a, b):
                ctx_ids, gidx, valid = scatter[k][g]
                for c in range(CT):
                    t, r2 = divmod(c, CPT)
                    sl, cf = r2 % NSL, r2 // NSL
                    # merged path stacks pair member cf at a G-row offset
                    # inside the strip
                    rb = sl * SLP + (cf * Gs[g] if geo["mrg"][g] else 0)
                    rows = blk[rb:rb + Gs[g],
                               g - a, t, cf, :]              # [G, S]
                    m = valid[c]
                    out_full[gidx[c][m]] = rows[m]

    if _trace:
        return out_full, res

    return out_full



# revision 25
# speedup vs baseline: 1.1169x; 1.1169x over previous
"""Trainium2 Bass kernel for ContextHyperLinearSSM.

Computes out[b,:] = x[b,:] @ (WA[context[b]] * adj_xx) + u[b,:] @ (WB[context[b]] * adj_xu)

Strategy: shard the CONTEXT axis across the 8 cores (64 contexts each).
The host groups samples by context, masks the weight banks with the
adjacency masks, and quantizes the weights to fp8-e3m4 (x2^6 scale; the
inverse 2^-6 is folded into the bf16 activations — both scalings are exact
powers of two, so the only quantization error is the e3m4 weight rounding,
measured at 1.3e-2 absmax/scale against the fp32 reference).  Each core
streams its 64 contexts' weights from HBM exactly once and runs 3
accumulating mixed-dtype matmuls (bf16 stationary x fp8 moving) per
context.  Each sample's row is computed by exactly one core, so the
host-side unshard is a pure scatter.

Contexts are globally sorted by sample count and dealt round-robin to the
cores, so every core sees the same per-group padded size G_g (required:
one SPMD program serves all cores) and the padding tracks the count
distribution.  ALL DMAs are issued up front, before any dependent work
lands on the two HWDGE rings' issuing engines (sync/scalar), so the
weight stream runs back-to-back at line rate (~420 GB/s measured).
Whole-group transfers stream fastest; only group 0 (pipeline fill) and
the last group (receipt pipelining at the tail) are split into halves.

Compute: where 2*G <= 32, a context pair (ci, CH+ci) merges into ONE
matmul per k-slot — stationary [x_ci | x_pair] (2G cols), moving
[W_ci[k] | W_pair[k]] (N=512); the two cross quadrants are garbage the
host never reads.  This halves the LDWEIGHTS/MATMUL count so even a
HAM-throttled (1.2 GHz) PE outruns the weight stream.  A short burst of
dummy matmuls on a zeroed tile pre-warms the HAM clock gate during the
DMA fill phase.  Pairs cycle the four 128x32 column tiles of the PE
array; each group accumulates in one PSUM bank.  One full-width copy per
group (vector/scalar alternating; the last group split across both)
drains PSUM into a bf16 staging tile, which is flushed by a few
consolidated multi-group DMAs (the final group as two free-dim halves,
each chained to its own drain half).
"""

import sys

sys.path.insert(0, "/opt/trn_rl_repo")

import ml_dtypes
import numpy as np

import concourse.bass as bass
import concourse.mybir as mybir
import concourse.tile as tile
from concourse import bacc
from concourse.bass_utils import run_bass_kernel_spmd

N_CORES = 8
CT = 8  # contexts per PSUM group
WSCALE = 64.0  # 2^6: weights *= WSCALE (into e3m4 range), acts /= WSCALE

BF16 = ml_dtypes.bfloat16
FP8 = ml_dtypes.float8_e3m4


def _install_profile_shim():
    """Register the NTFF profile hook that trn_boot skips when
    antenv.axon_hooks is missing from the image (profiling only)."""
    import types
    if "antenv.axon_hooks" in sys.modules:
        return
    try:
        from trn_agent_boot.trn_boot import _ntff_profile_via_ctypes
        hook = _ntff_profile_via_ctypes("/opt/axon/libaxon_pjrt.so")
    except Exception:
        hook = None
    mod = types.ModuleType("antenv.axon_hooks")
    mod.get_axon_ntff_profile_hook = lambda: hook
    mod.set_axon_ntff_profile_hook = lambda h: None
    sys.modules["antenv.axon_hooks"] = mod


def _geometry(S, A, Gs):
    """Static geometry shared by host and device.

    PSUM packing: context c of a group -> bank t=c//CPT, partition slot
    sl=(c%CPT)%NSL (SLP-aligned), free half cf=(c%CPT)//NSL.
    """
    HS = S // 128
    K = HS + 1
    CH = CT // 2
    WF = CH * K * S
    FF = max(1, min(CT, 512 // S))
    SLP = 32 if max(Gs) <= 32 else 64
    NSL = 128 // SLP
    CPT = min(CT, NSL * FF)
    T = -(-CT // CPT)
    assert T * CPT == CT
    NG = len(Gs)
    # out-staging splits: [0,NG/2), [NG/2,NG-1), [NG-1,NG) — early flushes
    # plus a tiny final one (groups are sorted largest-first)
    splits = [(0, NG // 2), (NG // 2, NG - 1), (NG - 1, NG)]
    splits = [(a, b) for a, b in splits if b > a]
    off_a = 0
    offs_a = []
    for G in Gs:
        offs_a.append(off_a)
        off_a += 2 * CH * K * G
    OW = T * FF * S  # staging elems per partition line per group
    # merged pair-matmul path: context pair (ci, CH+ci) stacks into one
    # 32-partition strip (2G rows) with an N=2S moving operand; needs the
    # 4 strips (NSL==CH) and both pair members inside one strip
    mrg = [T == 1 and NSL == CH and FF == 2 and 2 * G <= SLP for G in Gs]
    # compact per-group flush via strided-partition DMA: measured broken
    # on HW (wrong data + slower descriptors) — keep the padded
    # consolidated-split flushes instead
    cpk = False
    rows_o = [(2 * G if mrg[g] else G) if cpk else 128
              for g, G in enumerate(Gs)]
    offs_o = []
    off_o = 0
    for g in range(NG):
        offs_o.append(off_o)
        off_o += (CH * rows_o[g] if cpk else 128) * OW
    return dict(HS=HS, K=K, CH=CH, WF=WF, FF=FF, SLP=SLP, NSL=NSL,
                CPT=CPT, T=T, NG=NG, splits=splits, offs_a=offs_a,
                AL=off_a, OW=OW, OL=off_o, mrg=mrg, cpk=cpk,
                rows_o=rows_o, offs_o=offs_o)


def _build_program(S, A, Gs):
    """Build the per-core Bass program for per-group sizes Gs."""
    f32 = mybir.dt.float32
    bf16 = mybir.dt.bfloat16
    fp8 = mybir.dt.float8e3
    nc = bacc.Bacc("TRN2", target_bir_lowering=False)

    geo = _geometry(S, A, Gs)
    K, CH, WF, FF = geo["K"], geo["CH"], geo["WF"], geo["FF"]
    SLP, NSL, CPT, T = geo["SLP"], geo["NSL"], geo["CPT"], geo["T"]
    NG, OW = geo["NG"], geo["OW"]
    assert S % 128 == 0 and A == 128

    wts = nc.dram_tensor("wts", [NG, 128, 2 * WF], fp8,
                         kind="ExternalInput").ap()
    acts = nc.dram_tensor("acts", [128, geo["AL"]], bf16,
                          kind="ExternalInput").ap()
    out = nc.dram_tensor("out", [geo["OL"]], bf16,
                         kind="ExternalOutput").ap()

    with tile.TileContext(nc) as tc:
        with (
            tc.tile_pool(name="a", bufs=1) as apool,
            tc.tile_pool(name="w", bufs=NG) as wpool,
            tc.tile_pool(name="o", bufs=1) as opool,
            tc.tile_pool(name="d", bufs=1) as dpool,
            tc.tile_pool(name="psum", bufs=8, space="PSUM") as psum,
        ):
            # ---- ALL DMAs issued up front, before any dependent work
            # lands on the issuing engines: the two HWDGE rings then
            # stream back-to-back at line rate with no issue stalls.
            # Every transfer is split into free-dim halves, one half per
            # ring, in group-consumption order: the rings stay byte-
            # balanced, each group's halves land as a pair right before
            # the PE needs them, and each half's completion receipt
            # (~1.5us) hides behind the next half's transfer.
            at = apool.tile([128, geo["AL"]], bf16)
            nc.scalar.dma_start(at[:], acts[:])
            # shared output staging tile, flushed by consolidated DMAs
            os_t = opool.tile([128, NG * OW], bf16)

            # whole-group transfers stream fastest (~424 GB/s vs ~395 for
            # half-splits); halves only where latency matters — group 0
            # (pipeline fill) and the last group (its second half's
            # completion receipt hides behind the first's matmuls)
            wts_t = [wpool.tile([128, 2 * WF], fp8, tag="wt", name=f"wt_{g}")
                     for g in range(NG)]
            nc.sync.dma_start(wts_t[0][:, :WF], wts[0, :, :WF])
            nc.scalar.dma_start(wts_t[0][:, WF:], wts[0, :, WF:])
            for g in range(1, NG - 1):
                ring = nc.sync if g % 2 == 1 else nc.scalar
                ring.dma_start(wts_t[g][:], wts[g])
            nc.sync.dma_start(wts_t[NG - 1][:, :WF], wts[NG - 1, :, :WF])
            nc.scalar.dma_start(wts_t[NG - 1][:, WF:], wts[NG - 1, :, WF:])

            # ---- PE warm-up: the HAM clock gate holds the PE at 1.2 GHz
            # until it has been busy for a full ~3.4us window.  Dummy
            # matmuls on a zeroed tile (into the PSUM bank that group
            # NG-1 will reuse much later) run during the DMA fill phase
            # so the real matmuls start at 2.4 GHz.
            dt = dpool.tile([128, 512], bf16)
            nc.vector.memset(dt[:], 0.0)
            ps_dummy = psum.tile([128, 512], f32, tag="ps", name="ps_dummy")
            for i in range(9):
                nc.tensor.matmul(ps_dummy[:, :256], lhsT=dt[:, :128],
                                 rhs=dt[:, :256], start=True, stop=True)

            for g in range(NG):
                G = Gs[g]
                wt = wts_t[g]

                def views(c):
                    hf, ci = divmod(c, CH)
                    wv = wt[:, hf * WF + ci * K * S:
                            hf * WF + (ci + 1) * K * S] \
                        .rearrange("p (k s) -> p k s", k=K)
                    a0 = geo["offs_a"][g] + (hf * CH + ci) * K * G
                    av = at[:, a0:a0 + K * G] \
                        .rearrange("p (k g) -> p k g", k=K)
                    return wv, av

                ps_tiles = [psum.tile([128, FF * S], f32, tag="ps",
                                      name=f"ps_{g}_{t}")
                            for t in range(T)]
                if geo["mrg"][g]:
                    # merged path: context pair (ci, CH+ci) shares one
                    # matmul — stationary [x_ci | x_{CH+ci}] (2G cols),
                    # moving [W_ci[k] | W_{CH+ci}[k]] (2S cols).  The two
                    # cross quadrants of each [2G, 2S] output are garbage
                    # that the host never reads; the payoff is half the
                    # LDWEIGHTS/MATMUL instructions and N=512 streams, so
                    # even a cold (HAM-throttled) PE outruns the weight
                    # stream.  Pairs cycle the four column strips.
                    for ci in range(CH):
                        wv2 = wt[:, ci * K * 2 * S:(ci + 1) * K * 2 * S] \
                            .rearrange("p (k s) -> p k s", k=K)
                        a0 = geo["offs_a"][g] + ci * K * 2 * G
                        av2 = at[:, a0:a0 + K * 2 * G] \
                            .rearrange("p (k g) -> p k g", k=K)
                        pslice = ps_tiles[0][ci * SLP:ci * SLP + 2 * G, :]
                        for k in range(K):
                            nc.tensor.matmul(
                                pslice,
                                lhsT=av2[:, k, :],
                                rhs=wv2[:, k, :],
                                start=(k == 0), stop=(k == K - 1),
                                tile_position=(0, ci * SLP))
                else:
                    # consecutive contexts cycle the NSL column tiles of
                    # the PE array so their streams execute concurrently;
                    # each context's K accumulating matmuls stay adjacent
                    for c in range(CT):
                        wv, av = views(c)
                        t, r2 = divmod(c, CPT)
                        sl, cf = r2 % NSL, r2 // NSL
                        pslice = ps_tiles[t][sl * SLP:sl * SLP + G,
                                             cf * S:cf * S + S]
                        for k in range(K):
                            nc.tensor.matmul(
                                pslice,
                                lhsT=av[:, k, :],
                                rhs=wv[:, k, :],
                                start=(k == 0), stop=(k == K - 1),
                                tile_position=(0, sl * SLP))
                o0 = g * T * FF * S
                if g == NG - 1:
                    # tail group: split the latency-critical final drain
                    # into free-dim halves on vector+scalar in parallel
                    for t in range(T):
                        half = FF * S // 2
                        nc.vector.tensor_copy(
                            os_t[:, o0 + t * FF * S:
                                 o0 + t * FF * S + half],
                            ps_tiles[t][:, :half])
                        nc.scalar.copy(
                            os_t[:, o0 + t * FF * S + half:
                                 o0 + (t + 1) * FF * S],
                            ps_tiles[t][:, half:])
                else:
                    for t in range(T):
                        # engines alternate per group
                        eng = (nc.vector.tensor_copy if g % 2 == 0
                               else nc.scalar.copy)
                        eng(os_t[:, o0 + t * FF * S:
                                 o0 + (t + 1) * FF * S],
                            ps_tiles[t][:, :])
                if geo["cpk"]:
                    # compact per-group flush: only the valid strip rows
                    # ship to HBM (sync ring; issues overlap the stream)
                    R = geo["rows_o"][g]
                    src = os_t[:, o0:o0 + OW] \
                        .rearrange("(a b) w -> a b w", a=CH)[:, :R, :]
                    dst = out[geo["offs_o"][g]:
                              geo["offs_o"][g] + CH * R * OW] \
                        .rearrange("(a b w) -> a b w", a=CH, b=R)
                    nc.sync.dma_start(dst, src)

            if not geo["cpk"]:
                # consolidated output flushes: one full-partition DMA per
                # split; the final flush is two free-half DMAs chained to
                # their drain halves
                nsp = len(geo["splits"])
                for si, (a, b) in enumerate(geo["splits"]):
                    if si == nsp - 1 and T == 1 and FF == 2:
                        half = OW // 2
                        for cf in range(2):
                            dst = out[128 * a * OW + cf * 128 * half:
                                      128 * a * OW + (cf + 1) * 128 * half]
                            ring = nc.sync if cf == 0 else nc.scalar
                            ring.dma_start(
                                dst.rearrange("(p w) -> p w", p=128),
                                os_t[:, a * OW + cf * half:
                                     a * OW + (cf + 1) * half])
                    else:
                        dst = out[128 * a * OW: 128 * b * OW]
                        ring = nc.sync if si % 2 == 0 else nc.scalar
                        ring.dma_start(
                            dst.rearrange("(p w) -> p w", p=128),
                            os_t[:, a * OW:b * OW])

    nc.compile()
    return nc


def kernel(x, u, WA, WB, adj_xx, adj_xu, context, _trace=False):
    B, S = x.shape
    _, A = u.shape
    C = WA.shape[0]
    assert C % N_CORES == 0
    CP = C // N_CORES
    assert CP % CT == 0
    NG = CP // CT

    # ---- host-side shard: count-sorted contexts, dealt round-robin ----
    context = np.asarray(context)
    cnt = np.bincount(context, minlength=C)
    perm = np.argsort(-cnt, kind="stable")          # contexts by count desc
    # context at global rank r -> core r%8, position r//8; group = pos//CT.
    # All cores share one program, so G_g is set by the chunk's global max
    # count = count at rank g*CT*N_CORES.
    Gs = []
    for g in range(NG):
        m = int(cnt[perm[g * CT * N_CORES]])
        Gs.append(max(2, ((m + 1) // 2) * 2))

    geo = _geometry(S, A, Gs)
    HS, K, CH, WF = geo["HS"], geo["K"], geo["CH"], geo["WF"]
    FF, SLP, NSL, CPT, T, OW = (geo["FF"], geo["SLP"], geo["NSL"],
                                geo["CPT"], geo["T"], geo["OW"])

    order = np.argsort(context, kind="stable")
    starts = np.zeros(C + 1, np.int64)
    starts[1:] = np.cumsum(cnt)

    def group_rows(ctx_ids, G):
        """gidx [len,G] sample indices (clamped) + valid mask."""
        j = np.arange(G)
        cc = cnt[ctx_ids][:, None]
        valid = j[None, :] < cc
        pos = starts[ctx_ids][:, None] + np.minimum(j[None, :],
                                                    np.maximum(cc - 1, 0))
        return order[pos], valid

    inv = np.float32(1.0 / WSCALE)
    x = np.asarray(x, np.float32) * inv
    u = np.asarray(u, np.float32) * inv

    # pre-mask the weight banks, scale into e3m4 range, quantize on host
    Am = (np.asarray(WA, np.float32) * np.float32(WSCALE)
          * np.asarray(adj_xx, np.float32)).astype(FP8)    # [C, S, S]
    Bm = (np.asarray(WB, np.float32) * np.float32(WSCALE)
          * np.asarray(adj_xu, np.float32)).astype(FP8)    # [C, A, S]

    in_maps = []
    scatter = []   # per core: list of (ctx_ids, gidx, valid) per group
    for k in range(N_CORES):
        wblob = np.empty((NG, 128, 2 * WF), FP8)
        ablob = np.zeros((128, geo["AL"]), BF16)
        sc = []
        for g in range(NG):
            G = Gs[g]
            ctx_ids = perm[(g * CT + np.arange(CT)) * N_CORES + k]
            gidx, valid = group_rows(ctx_ids, G)           # [CT, G]
            sc.append((ctx_ids, gidx, valid))
            XpT = x[gidx].transpose(0, 2, 1).astype(BF16)  # [CT, S, G]
            UpT = u[gidx].transpose(0, 2, 1).astype(BF16)  # [CT, A, G]
            # per-partition element order: legacy (hf, ci, k, s) vs
            # merged (ci, k, hf, s); both views expose [hf,ci,128,k,s]
            if geo["mrg"][g]:
                wb = wblob[g].reshape(128, CH, K, 2, S) \
                    .transpose(3, 1, 0, 2, 4)
            else:
                wb = wblob[g].reshape(128, 2, CH, K, S) \
                    .transpose(1, 2, 0, 3, 4)
            wb[..., 0, :] = Bm[ctx_ids].reshape(2, CH, 128, S)
            wb[..., 1:, :] = Am[ctx_ids].reshape(2, CH, HS, 128, S) \
                .transpose(0, 1, 3, 2, 4)
            asl = ablob[:, geo["offs_a"][g]:
                        geo["offs_a"][g] + 2 * CH * K * G]
            if geo["mrg"][g]:
                A3 = asl.reshape(128, CH, K, 2, G).transpose(3, 1, 2, 0, 4)
            else:
                A3 = asl.reshape(128, 2, CH, K, G).transpose(1, 2, 3, 0, 4)
            A3[:, :, 0] = UpT.reshape(2, CH, 128, G)
            A3[:, :, 1:] = XpT.reshape(2, CH, HS, 128, G)
        in_maps.append({"wts": wblob, "acts": ablob})
        scatter.append(sc)

    if _trace:
        _install_profile_shim()
    nc = _build_program(S, A, Gs)
    res = run_bass_kernel_spmd(nc, in_maps, core_ids=list(range(N_CORES)),
                               trace=_trace)

    out_full = np.zeros((B, S), np.float32)
    for k, r in enumerate(res.results):
        v = np.asarray(r["out"]).astype(np.float32)
        if geo["cpk"]:
            # compact per-group blocks: [CH strips, rows_o, FF*S]
            for g in range(NG):
                R = geo["rows_o"][g]
                blk = v[geo["offs_o"][g]:
                        geo["offs_o"][g] + CH * R * geo["OW"]] \
                    .reshape(CH, R, FF, S)
                ctx_ids, gidx, valid = scatter[k][g]
                for c in range(CT):
                    sl, cf = c % NSL, c // NSL
                    rb = cf * Gs[g] if geo["mrg"][g] else 0
                    rows = blk[sl, rb:rb + Gs[g], cf, :]     # [G, S]
                    m = valid[c]
                    out_full[gidx[c][m]] = rows[m]
            continue
        for si, (a, b) in enumerate(geo["splits"]):
            nsp = len(geo["splits"])
            if si == nsp - 1 and T == 1 and FF == 2:
                half = OW // 2
                h = [v[128 * a * OW + cf * 128 * half:
                       128 * a * OW + (cf + 1) * 128 * half]
                     .reshape(128, 1, T, 1, S) for cf in range(2)]
                blk = np.concatenate(h, axis=3)   # [128, 1, T, FF, S]
            else:
                blk = v[128 * a * OW: 128 * b * OW] \
                    .reshape(128, b - a, T, FF, S)
            for g in range(a, b):
                ctx_ids, gidx, valid = scatter[k][g]
                for c in range(CT):
                    t, r2 = divmod(c, CPT)
                    sl, cf = r2 % NSL, r2 // NSL
                    # merged path stacks pair member cf at a G-row offset
                    # inside the strip
                    rb = sl * SLP + (cf * Gs[g] if geo["mrg"][g] else 0)
                    rows = blk[rb:rb + Gs[g],
                               g - a, t, cf, :]              # [G, S]
                    m = valid[c]
                    out_full[gidx[c][m]] = rows[m]

    if _trace:
        return out_full, res

    return out_full

